# revision 1
# baseline (speedup 1.0000x reference)
"""Trainium2 Bass kernel for a Linformer transformer block (nn_Block).

Shapes (hardcoded): B=2, N=8192, C=768, H=12, D=64, K=256, HID=3072.
Sharding: 8 cores, data-parallel over tokens (2048 tokens/core, batch-major:
cores 0-3 hold batch 0, cores 4-7 batch 1). The Linformer K/V projections
reduce over the full sequence, so each core computes partials over its token
shard and a grouped AllReduce (replica groups [0-3], [4-7]) combines them.

Matmul dtypes: float32r (full-rate fp32 mode) for QKV/attention/proj,
bf16 for the MLP. LayerNorm/softmax/residuals in fp32.
"""

import sys
sys.path.insert(0, "/opt/trn_rl_repo")

import numpy as np
import ml_dtypes

import concourse.bass as bass
import concourse.mybir as mybir
import concourse.tile as tile
from concourse import bacc
from concourse.bass_utils import run_bass_kernel_spmd
from concourse.masks import make_identity

F32 = mybir.dt.float32
F32R = mybir.dt.float32r
BF16 = mybir.dt.bfloat16
AF = mybir.ActivationFunctionType
ALU = mybir.AluOpType
AX = mybir.AxisListType

B, N, C = 2, 8192, 768
H, K = 12, 256
D = C // H                 # 64
HID = 4 * C                # 3072
EPS = 1e-6
NCORES = 8
T = (B * N) // NCORES      # 2048 tokens per core
NT = T // 128              # 16 token tiles
NG = T // 512              # 4 token groups
CK = C // 128              # 6 contraction chunks of C
HC = HID // 128            # 24 hidden chunks
KC = K // 128              # 2 kk chunks
SCALE = float(D) ** -0.5   # 0.125

_CACHE = {}


def _ln_stats(nc, pool, xt, scratch, tag):
    """LayerNorm stats for a (128, C) fp32 tile -> (rstd, -mu*rstd) (128,1)."""
    NSUB = 3  # 768 = 3 x 256 (BN_STATS_FMAX=512, gcd=256)
    stats = pool.tile([128, NSUB, 6], F32, tag=f"{tag}_stats")
    xv = xt.rearrange("p (j s) -> p j s", j=NSUB)
    for j in range(NSUB):
        nc.vector.bn_stats(stats[:, j, :], xv[:, j, :])
    mv = pool.tile([128, 2], F32, tag=f"{tag}_mv")
    nc.vector.bn_aggr(mv[:], stats[:])
    var = pool.tile([128, 1], F32, tag=f"{tag}_var")
    nc.vector.tensor_scalar_add(var[:], mv[:, 1:2], EPS)
    std = pool.tile([128, 1], F32, tag=f"{tag}_std")
    nc.scalar.activation(std[:], var[:], AF.Sqrt)
    rstd = pool.tile([128, 1], F32, tag=f"{tag}_rstd")
    nc.vector.reciprocal(rstd[:], std[:])
    nmr = pool.tile([128, 1], F32, tag=f"{tag}_nmr")
    nc.vector.scalar_tensor_tensor(nmr[:], mv[:, 0:1], -1.0, rstd[:],
                                   op0=ALU.mult, op1=ALU.mult)
    return rstd, nmr


def build(ln1_triv, ln2_triv, kvb_zero, projb_zero, fc2b_zero):
    nc = bacc.Bacc("TRN2", target_bir_lowering=False, debug=False,
                   enable_asserts=True, num_devices=NCORES)

    x_s = nc.dram_tensor("x_s", [T, C], F32, kind="ExternalInput").ap()
    qkv_w = nc.dram_tensor("qkv_w", [C, 3 * C], F32R, kind="ExternalInput").ap()
    qkv_b = nc.dram_tensor("qkv_b", [3 * C], F32, kind="ExternalInput").ap()
    Ek_s = nc.dram_tensor("Ek_s", [T, K], F32R, kind="ExternalInput").ap()
    Ev_s = nc.dram_tensor("Ev_s", [T, K], F32R, kind="ExternalInput").ap()
    proj_w = nc.dram_tensor("proj_w", [C, C], F32R, kind="ExternalInput").ap()
    proj_b = nc.dram_tensor("proj_b", [C], F32, kind="ExternalInput").ap()
    fc1_w = nc.dram_tensor("fc1_w", [HC // 2, 128, 2 * C], BF16,
                           kind="ExternalInput").ap()
    fc1_b = nc.dram_tensor("fc1_b", [HID], F32, kind="ExternalInput").ap()
    fc2_w = nc.dram_tensor("fc2_w", [HID, C], BF16, kind="ExternalInput").ap()
    fc2_b = nc.dram_tensor("fc2_b", [C], F32, kind="ExternalInput").ap()
    ln1_w = nc.dram_tensor("ln1_w", [1, C], F32, kind="ExternalInput").ap()
    ln1_b = nc.dram_tensor("ln1_b", [1, C], F32, kind="ExternalInput").ap()
    ln2_w = nc.dram_tensor("ln2_w", [1, C], F32, kind="ExternalInput").ap()
    ln2_b = nc.dram_tensor("ln2_b", [1, C], F32, kind="ExternalInput").ap()
    out = nc.dram_tensor("out", [T, C], F32, kind="ExternalOutput").ap()

    qkv_w_r = qkv_w.rearrange("(k p) n -> p k n", p=128)    # (128, 6, 2304)
    Ek_r = Ek_s.rearrange("(i p) k -> p i k", p=128)        # (128, 16, 256)
    Ev_r = Ev_s.rearrange("(i p) k -> p i k", p=128)
    proj_w_r = proj_w.rearrange("(k p) n -> p k n", p=128)  # (128, 6, 768)
    fc2_w_r = fc2_w.rearrange("(k p) n -> p k n", p=128)    # (128, 24, 768)

    with tile.TileContext(nc) as tc:
      with tc.tile_pool(name="const", bufs=1) as constp, \
           tc.tile_pool(name="dram", bufs=1, space="DRAM") as dram:
        ident = constp.tile([128, 128], F32, tag="ident")
        make_identity(nc, ident)
        ones_r = constp.tile([128, 1], F32R, tag="ones_r")
        nc.scalar.activation(ones_r[:], ident[:, 0:1], AF.Copy,
                             bias=1.0, scale=0.0)
        qkvb = constp.tile([128, 18], F32, tag="qkvb")
        nc.sync.dma_start(qkvb[:], qkv_b.rearrange("(m p) -> p m", p=128))
        qb_scaled = constp.tile([128, 6], F32, tag="qb_scaled")
        nc.vector.tensor_scalar_mul(qb_scaled[:], qkvb[:, 0:6], SCALE)
        fc1b = constp.tile([128, 24], F32, tag="fc1b")
        nc.sync.dma_start(fc1b[:], fc1_b.rearrange("(m p) -> p m", p=128))

        def bcast_row(name, src_ap, width):
            row = constp.tile([1, width], F32, tag=f"{name}_row")
            nc.sync.dma_start(row[:], src_ap)
            bc = constp.tile([128, width], F32, tag=f"{name}_bc")
            nc.gpsimd.partition_broadcast(bc[:], row[:])
            return bc

        ln1w_bc = ln1b_bc = ln2w_bc = ln2b_bc = None
        kvb_bc = projb_bc = fc2b_bc = None
        if not ln1_triv:
            ln1w_bc = bcast_row("ln1w", ln1_w[:], C)
            ln1b_bc = bcast_row("ln1b", ln1_b[:], C)
        if not ln2_triv:
            ln2w_bc = bcast_row("ln2w", ln2_w[:], C)
            ln2b_bc = bcast_row("ln2b", ln2_b[:], C)
        if not kvb_zero:
            kvb_bc = bcast_row("kvb", qkv_b[None, C:3 * C], 2 * C)
        if not projb_zero:
            projb_bc = bcast_row("projb", proj_b[None, :], C)
        if not fc2b_zero:
            fc2b_bc = bcast_row("fc2b", fc2_b[None, :], C)

        ar_in = dram.tile([128, 2, 1536], F32)
        ar_out = dram.tile([128, 2, 1536], F32)
        qT_dram = dram.tile([NG, 128, CK, 512], F32)

        if True:
          # ===== Stage A1: LN1, h1T, k/v, kv-proj psum accumulation ========
          with tc.tile_pool(name="Apool", bufs=1) as Ap:
            h1T = Ap.tile([128, CK, T], F32R, tag="h1T")
            qkvw_sb = Ap.tile([128, CK, 3 * C], F32R, tag="qkvw")
            nc.sync.dma_start(qkvw_sb[:], qkv_w_r)

            with tc.tile_pool(name="A1", bufs=2) as wk, \
                 tc.tile_pool(name="psA", bufs=2, space="PSUM") as psA:
                # ---- pass 1: LN1 + h1T + k-projection partials ----
                with tc.tile_pool(name="psK", bufs=1, space="PSUM") as psK:
                    kacc = psK.tile([128, 1536], F32, tag="kacc")
                    for i in range(NT):
                        xt = wk.tile([128, C], F32, tag="xt")
                        nc.sync.dma_start(xt[:], x_s[i * 128:(i + 1) * 128, :])
                        h1 = wk.tile([128, C], F32, tag="h1")
                        rstd, nmr = _ln_stats(nc, wk, xt, h1, "ln1")
                        nc.vector.tensor_scalar(h1[:], xt[:], rstd[:], nmr[:],
                                                op0=ALU.mult, op1=ALU.add)
                        if ln1w_bc is not None:
                            nc.vector.tensor_mul(h1[:], h1[:], ln1w_bc[:])
                            nc.vector.tensor_add(h1[:], h1[:], ln1b_bc[:])
                        for k in range(CK):
                            tp = psA.tile([128, 128], F32, tag="tp")
                            nc.tensor.transpose(
                                tp[:], h1[:, k * 128:(k + 1) * 128], ident[:])
                            nc.vector.tensor_copy(
                                h1T[:, k, i * 128:(i + 1) * 128], tp[:])
                        kvk = wk.tile([128, 768], F32R, tag="kvk")
                        for lo, hi in ((0, 512), (512, 768)):
                            mmp = psA.tile([128, 512], F32, tag="kvmm")
                            for k in range(CK):
                                nc.tensor.matmul(
                                    mmp[:, 0:hi - lo],
                                    h1T[:, k, i * 128:(i + 1) * 128],
                                    qkvw_sb[:, k, C + lo:C + hi],
                                    start=(k == 0), stop=(k == CK - 1))
                            nc.vector.tensor_copy(kvk[:, lo:hi],
                                                  mmp[:, 0:hi - lo])
                        if kvb_bc is not None:
                            nc.vector.tensor_add(kvk[:], kvk[:],
                                                 kvb_bc[:, 0:C])
                        Ekc = wk.tile([128, K], F32R, tag="Ekc")
                        nc.sync.dma_start(Ekc[:], Ek_r[:, i, :])
                        st = (i == 0)
                        sp = (i == NT - 1)
                        for kc in range(KC):
                            for lo, hi in (((0, 512), (512, 768)) if kc == 0
                                           else ((768, 1024), (1024, 1536))):
                                nc.tensor.matmul(
                                    kacc[:, lo:hi],
                                    Ekc[:, kc * 128:(kc + 1) * 128],
                                    kvk[:, lo - kc * 768:hi - kc * 768],
                                    start=st, stop=sp)
                    kacc_sb = wk.tile([128, 1536], F32, tag="kacc_sb")
                    nc.scalar.activation(kacc_sb[:], kacc[:], AF.Copy)
                    nc.sync.dma_start(ar_in[:, 0, :], kacc_sb[:])
                # ---- pass 2: v-projection partials (h1T already built) ----
                with tc.tile_pool(name="psV", bufs=1, space="PSUM") as psV:
                    vacc = psV.tile([128, 1536], F32, tag="vacc")
                    for i in range(NT):
                        kvv = wk.tile([128, 768], F32R, tag="kvk")
                        for lo, hi in ((0, 512), (512, 768)):
                            mmp = psA.tile([128, 512], F32, tag="kvmm")
                            for k in range(CK):
                                nc.tensor.matmul(
                                    mmp[:, 0:hi - lo],
                                    h1T[:, k, i * 128:(i + 1) * 128],
                                    qkvw_sb[:, k, 2 * C + lo:2 * C + hi],
                                    start=(k == 0), stop=(k == CK - 1))
                            nc.vector.tensor_copy(kvv[:, lo:hi],
                                                  mmp[:, 0:hi - lo])
                        if kvb_bc is not None:
                            nc.vector.tensor_add(kvv[:], kvv[:],
                                                 kvb_bc[:, C:2 * C])
                        Evc = wk.tile([128, K], F32R, tag="Ekc")
                        nc.sync.dma_start(Evc[:], Ev_r[:, i, :])
                        st = (i == 0)
                        sp = (i == NT - 1)
                        for kc in range(KC):
                            for lo, hi in (((0, 512), (512, 768)) if kc == 0
                                           else ((768, 1024), (1024, 1536))):
                                nc.tensor.matmul(
                                    vacc[:, lo:hi],
                                    Evc[:, kc * 128:(kc + 1) * 128],
                                    kvv[:, lo - kc * 768:hi - kc * 768],
                                    start=st, stop=sp)
                    vacc_sb = wk.tile([128, 1536], F32, tag="kacc_sb")
                    nc.scalar.activation(vacc_sb[:], vacc[:], AF.Copy)
                    nc.sync.dma_start(ar_in[:, 1, :], vacc_sb[:])

            nc.gpsimd.collective_compute(
                "AllReduce", ALU.add,
                replica_groups=[[0, 1, 2, 3], [4, 5, 6, 7]],
                ins=[ar_in.opt()], outs=[ar_out.opt()])

            # ===== Stage A2: qT (overlaps the AllReduce) ====================
            with tc.tile_pool(name="qTev", bufs=2) as qTev, \
                 tc.tile_pool(name="psB", bufs=2, space="PSUM") as psB:
                for g in range(NG):
                    qTs = qTev.tile([128, CK, 512], F32, tag="qTs")
                    for m in range(CK):
                        qp = psB.tile([128, 512], F32, tag="qTps")
                        for k in range(CK):
                            nc.tensor.matmul(
                                qp[:], qkvw_sb[:, k, m * 128:(m + 1) * 128],
                                h1T[:, k, g * 512:(g + 1) * 512],
                                start=(k == 0), stop=(k == CK - 1))
                        nc.scalar.activation(qTs[:, m, :], qp[:], AF.Identity,
                                             scale=SCALE,
                                             bias=qb_scaled[:, m:m + 1])
                    nc.sync.dma_start(qT_dram[g], qTs[:])
          # Apool (h1T, qkvw) closes here

          # ===== post-AR: kv_r load, k_projT transposes ====================
          with tc.tile_pool(name="kvp", bufs=1) as kvp:
            v_r = kvp.tile([128, 1536], F32R, tag="v_r")
            nc.sync.dma_start(v_r[:], ar_out[:, 1, :].bitcast(F32R))
            kT_sb = kvp.tile([128, CK, K], F32R, tag="kT")
            with tc.tile_pool(name="kfp", bufs=1) as kfp, \
                 tc.tile_pool(name="psT", bufs=2, space="PSUM") as psT:
                kf = kfp.tile([128, 1536], F32, tag="kf")
                nc.sync.dma_start(kf[:], ar_out[:, 0, :])
                for kc in range(KC):
                    for m in range(CK):
                        tpk = psT.tile([128, 128], F32, tag="tpk")
                        nc.tensor.transpose(
                            tpk[:],
                            kf[:, kc * 768 + m * 128:kc * 768 + (m + 1) * 128],
                            ident[:])
                        nc.vector.tensor_copy(
                            kT_sb[:, m, kc * 128:(kc + 1) * 128], tpk[:])

            # ===== Attention + proj + LN2 + MLP ==============================
            with tc.tile_pool(name="attn", bufs=2) as at, \
                 tc.tile_pool(name="attn1", bufs=1) as at1, \
                 tc.tile_pool(name="attn3", bufs=3) as at3, \
                 tc.tile_pool(name="prj", bufs=2) as pj, \
                 tc.tile_pool(name="prj1", bufs=1) as pj1, \
                 tc.tile_pool(name="prjx", bufs=2) as pjx, \
                 tc.tile_pool(name="mlp", bufs=2) as ml, \
                 tc.tile_pool(name="mlp1", bufs=1) as ml1, \
                 tc.tile_pool(name="psC", bufs=1, space="PSUM") as psC, \
                 tc.tile_pool(name="psL", bufs=2, space="PSUM") as psL, \
                 tc.tile_pool(name="psCo", bufs=2, space="PSUM") as psCo, \
                 tc.tile_pool(name="psF", bufs=2, space="PSUM") as psF, \
                 tc.tile_pool(name="psD", bufs=1, space="PSUM") as psD:
                pw = pj1.tile([128, CK, C], F32R, tag="pw")
                nc.sync.dma_start(pw[:], proj_w_r)

                for g in range(NG):
                    # ---- attention for token group g ----
                    qTg = at1.tile([128, CK, 512], F32R, tag="qTg")
                    nc.sync.dma_start(qTg[:], qT_dram[g].bitcast(F32R))
                    oT = at.tile([128, CK, 512], F32R, tag="oT")
                    for ph in range(H // 2):
                        eTs = []
                        for sub in range(2):
                            h = 2 * ph + sub
                            off = 64 * (h % 2)
                            ch = h // 2
                            eT = at.tile([128, KC, 512], F32R, tag=f"eT{ph % 2}")
                            for kc in range(KC):
                                lg = psL.tile([128, 512], F32, tag="lg")
                                nc.tensor.matmul(
                                    lg[:],
                                    kT_sb[off:off + 64, ch,
                                          kc * 128:(kc + 1) * 128],
                                    qTg[off:off + 64, ch, :],
                                    start=True, stop=True,
                                    tile_position=(off, 0))
                                nc.scalar.activation(eT[:, kc, :], lg[:],
                                                     AF.Exp)
                            dnr = at3.tile([128, KC, 512], F32, tag="dnr")
                            nc.gpsimd.partition_all_reduce(
                                dnr[:], eT[:].bitcast(F32), channels=128,
                                reduce_op=bass.bass_isa.ReduceOp.add)
                            dnv = at.tile([128, 512], F32, tag="dnv")
                            nc.gpsimd.tensor_add(dnv[:], dnr[:, 0, :],
                                                 dnr[:, 1, :])
                            rcb = at3.tile([128, 512], F32, tag="rcb")
                            nc.vector.reciprocal(rcb[:], dnv[:])
                            eTs.append((eT, rcb))
                        # o^T for the head pair: lhsT covers both heads' v
                        # columns; each rhs makes one head's 64 rows valid.
                        pa = psCo.tile([128, 512], F32, tag="oTps")
                        pb = psCo.tile([128, 512], F32, tag="oTps")
                        for kc in range(KC):
                            vsl = v_r[:, kc * 768 + ph * 128:
                                      kc * 768 + (ph + 1) * 128]
                            nc.tensor.matmul(pa[:], vsl, eTs[0][0][:, kc, :],
                                             start=(kc == 0),
                                             stop=(kc == KC - 1))
                            nc.tensor.matmul(pb[:], vsl, eTs[1][0][:, kc, :],
                                             start=(kc == 0),
                                             stop=(kc == KC - 1))
                        nc.vector.tensor_mul(oT[0:64, ph, :], pa[0:64, :],
                                             eTs[0][1][0:64, :])
                        nc.vector.tensor_mul(oT[64:128, ph, :], pb[64:128, :],
                                             eTs[1][1][64:128, :])

                    # ---- proj + residual + LN2 + h2T for group g ----
                    h2T = pj.tile([128, CK, 512], BF16, tag="h2T")
                    x2g = pjx.tile([128, 4, C], F32, tag="x2g")
                    for ms in range(4):
                        r0 = g * 512 + ms * 128
                        xr = pj.tile([128, C], F32, tag="xr")
                        nc.sync.dma_start(xr[:], x_s[r0:r0 + 128, :])
                        for cs in range(2):
                            pp = psC.tile([128, 384], F32, tag="pjtp")
                            for k in range(CK):
                                nc.tensor.matmul(
                                    pp[:], oT[:, k, ms * 128:(ms + 1) * 128],
                                    pw[:, k, cs * 384:(cs + 1) * 384],
                                    start=(k == 0), stop=(k == CK - 1))
                            nc.vector.tensor_add(
                                x2g[:, ms, cs * 384:(cs + 1) * 384], pp[:],
                                xr[:, cs * 384:(cs + 1) * 384])
                        if projb_bc is not None:
                            nc.vector.tensor_add(x2g[:, ms, :], x2g[:, ms, :],
                                                 projb_bc[:])
                    # LN2 stats batched (one ACT table visit for all sqrt)
                    NSUB = 3
                    lnv = []
                    for ms in range(4):
                        stats = pj.tile([128, NSUB, 6], F32, tag=f"l2s{ms}")
                        xv = x2g[:, ms, :].rearrange("p (j s) -> p j s", j=NSUB)
                        for j in range(NSUB):
                            nc.vector.bn_stats(stats[:, j, :], xv[:, j, :])
                        mv = pj.tile([128, 2], F32, tag=f"l2mv{ms}")
                        nc.vector.bn_aggr(mv[:], stats[:])
                        var = pj.tile([128, 1], F32, tag=f"l2var{ms}")
                        nc.vector.tensor_scalar_add(var[:], mv[:, 1:2], EPS)
                        lnv.append((mv, var))
                    stds = []
                    for ms in range(4):
                        std = pj.tile([128, 1], F32, tag=f"l2std{ms}")
                        nc.scalar.activation(std[:], lnv[ms][1][:], AF.Sqrt)
                        stds.append(std)
                    for ms in range(4):
                        mv, var = lnv[ms]
                        rstd = pj.tile([128, 1], F32, tag=f"l2r{ms}")
                        nc.vector.reciprocal(rstd[:], stds[ms][:])
                        nmr = pj.tile([128, 1], F32, tag=f"l2n{ms}")
                        nc.vector.scalar_tensor_tensor(
                            nmr[:], mv[:, 0:1], -1.0, rstd[:],
                            op0=ALU.mult, op1=ALU.mult)
                        h2 = pj.tile([128, C], F32, tag="h2")
                        nc.vector.tensor_scalar(h2[:], x2g[:, ms, :],
                                                rstd[:], nmr[:],
                                                op0=ALU.mult, op1=ALU.add)
                        if ln2w_bc is not None:
                            nc.vector.tensor_mul(h2[:], h2[:], ln2w_bc[:])
                            nc.vector.tensor_add(h2[:], h2[:], ln2b_bc[:])
                        for k in range(CK):
                            tp2 = psC.tile([128, 128], F32, tag="pjtp")
                            nc.tensor.transpose(
                                tp2[:], h2[:, k * 128:(k + 1) * 128], ident[:])
                            nc.vector.tensor_copy(
                                h2T[:, k, ms * 128:(ms + 1) * 128], tp2[:])

                    # ---- MLP for group g (bf16, weights streamed) ----
                    gT = ml1.tile([128, HC, 512], BF16, tag="gT")
                    for hp in range(HC // 2):
                        f1c = ml.tile([128, 2, CK, 128], BF16, tag="f1c")
                        nc.sync.dma_start(
                            f1c[:], fc1_w[hp].rearrange(
                                "p (s k c) -> p s k c", s=2, k=CK))
                        for s in range(2):
                            hc = 2 * hp + s
                            fp = psF.tile([128, 512], F32, tag="fp")
                            for k in range(CK):
                                nc.tensor.matmul(fp[:], f1c[:, s, k, :],
                                                 h2T[:, k, :],
                                                 start=(k == 0),
                                                 stop=(k == CK - 1))
                            nc.scalar.activation(gT[:, hc, :], fp[:], AF.Gelu,
                                                 bias=fc1b[:, hc:hc + 1])
                    for cs in range(2):
                        f2h = ml1.tile([128, HC, 384], BF16, tag="f2h")
                        nc.sync.dma_start(
                            f2h[:], fc2_w_r[:, :, cs * 384:(cs + 1) * 384])
                        for ms in range(4):
                            r0 = g * 512 + ms * 128
                            op = psD.tile([128, 384], F32, tag="op")
                            for hc in range(HC):
                                nc.tensor.matmul(
                                    op[:], gT[:, hc, ms * 128:(ms + 1) * 128],
                                    f2h[:, hc, :],
                                    start=(hc == 0), stop=(hc == HC - 1))
                            oth = ml.tile([128, 384], F32, tag="oth")
                            nc.vector.tensor_add(
                                oth[:], op[:],
                                x2g[:, ms, cs * 384:(cs + 1) * 384])
                            if fc2b_bc is not None:
                                nc.vector.tensor_add(
                                    oth[:], oth[:],
                                    fc2b_bc[:, cs * 384:(cs + 1) * 384])
                            nc.sync.dma_start(
                                out[r0:r0 + 128, cs * 384:(cs + 1) * 384],
                                oth[:])

    nc.compile()
    return nc


def kernel(**inputs):
    x = np.ascontiguousarray(np.asarray(inputs["x"], dtype=np.float32))
    qkv_w = np.ascontiguousarray(np.asarray(inputs["qkv_w"], dtype=np.float32))
    qkv_b = np.ascontiguousarray(np.asarray(inputs["qkv_b"], dtype=np.float32))
    Ek = np.ascontiguousarray(np.asarray(inputs["Ek"], dtype=np.float32))
    Ev = np.ascontiguousarray(np.asarray(inputs["Ev"], dtype=np.float32))
    proj_w = np.ascontiguousarray(np.asarray(inputs["proj_w"], dtype=np.float32))
    proj_b = np.ascontiguousarray(np.asarray(inputs["proj_b"], dtype=np.float32))
    fc1_w = np.asarray(inputs["fc1_w"], dtype=np.float32)
    fc1_b = np.ascontiguousarray(np.asarray(inputs["fc1_b"], dtype=np.float32))
    fc2_w = np.asarray(inputs["fc2_w"], dtype=np.float32)
    fc2_b = np.ascontiguousarray(np.asarray(inputs["fc2_b"], dtype=np.float32))
    ln1_w = np.asarray(inputs["ln1_w"], dtype=np.float32)
    ln1_b = np.asarray(inputs["ln1_b"], dtype=np.float32)
    ln2_w = np.asarray(inputs["ln2_w"], dtype=np.float32)
    ln2_b = np.asarray(inputs["ln2_b"], dtype=np.float32)

    ln1_triv = bool(np.all(ln1_w == 1.0) and np.all(ln1_b == 0.0))
    ln2_triv = bool(np.all(ln2_w == 1.0) and np.all(ln2_b == 0.0))
    kvb_zero = bool(np.all(qkv_b[C:] == 0.0))
    projb_zero = bool(np.all(proj_b == 0.0))
    fc2b_zero = bool(np.all(fc2_b == 0.0))

    key = (ln1_triv, ln2_triv, kvb_zero, projb_zero, fc2b_zero)
    if key not in _CACHE:
        _CACHE[key] = build(*key)
    nc = _CACHE[key]

    # [HC//2, 128, 2*C]: [hp, p, s*C + k*128 + j] = fc1_w[k*128+p, (2hp+s)*128+j]
    f1 = fc1_w.astype(ml_dtypes.bfloat16)
    f1 = f1.reshape(CK, 128, HC // 2, 2, 128)
    fc1_wb = np.ascontiguousarray(
        f1.transpose(2, 1, 3, 0, 4).reshape(HC // 2, 128, 2 * C))
    fc2_wb = np.ascontiguousarray(fc2_w.astype(ml_dtypes.bfloat16))

    xf = x.reshape(B * N, C)
    in_maps = []
    for c in range(NCORES):
        pos0 = (c % 4) * T
        in_maps.append({
            "x_s": np.ascontiguousarray(xf[c * T:(c + 1) * T]),
            "qkv_w": qkv_w,
            "qkv_b": qkv_b,
            "Ek_s": np.ascontiguousarray(Ek[pos0:pos0 + T]),
            "Ev_s": np.ascontiguousarray(Ev[pos0:pos0 + T]),
            "proj_w": proj_w,
            "proj_b": proj_b,
            "fc1_w": fc1_wb,
            "fc1_b": fc1_b,
            "fc2_w": fc2_wb,
            "fc2_b": fc2_b,
            "ln1_w": np.ascontiguousarray(ln1_w.reshape(1, C)),
            "ln1_b": np.ascontiguousarray(ln1_b.reshape(1, C)),
            "ln2_w": np.ascontiguousarray(ln2_w.reshape(1, C)),
            "ln2_b": np.ascontiguousarray(ln2_b.reshape(1, C)),
        })

    import os
    trace = bool(os.environ.get("NN_BLOCK_TRACE"))
    res = run_bass_kernel_spmd(nc, in_maps, core_ids=list(range(NCORES)),
                               trace=trace)
    global LAST_RESULT
    LAST_RESULT = res
    outs = np.concatenate([res.results[c]["out"] for c in range(NCORES)],
                          axis=0)
    return outs.reshape(B, N, C)


LAST_RESULT = None



# revision 11
# speedup vs baseline: 1.6241x; 1.6241x over previous
"""Trainium2 Bass kernel for a Linformer transformer block (nn_Block).

Shapes (hardcoded): B=2, N=8192, C=768, H=12, D=64, K=256, HID=3072.
Sharding: 8 cores, data-parallel over tokens (2048 tokens/core, batch-major:
cores 0-3 hold batch 0, cores 4-7 batch 1). The Linformer K/V projections
reduce over the full sequence, so each core computes partials over its token
shard and a grouped AllReduce (replica groups [0-3], [4-7]) combines them.

Precision plan: the two big GEMM groups (QKV and the MLP) run in fp8 e4m3
with DoubleRow perf mode (two 128-deep contraction chunks per instruction,
4x bf16 MAC rate); their weights are pre-scaled by 128 on the host so they
sit in fp8's normal range. Everything downstream of the QKV projection
(K/V Linformer projection, attention logits, softmax, attention-value,
output projection) runs in bf16 to keep quantization noise inside the 2e-2
gate. LayerNorm, softmax normalization and residuals are fp32. h1/h2
transposes go through the DMA XBAR (bf16) instead of the PE array.
"""

import sys
sys.path.insert(0, "/opt/trn_rl_repo")

import numpy as np
import ml_dtypes

import concourse.bass as bass
import concourse.mybir as mybir
import concourse.tile as tile
from concourse import bacc
from concourse.bass_utils import run_bass_kernel_spmd
from concourse.masks import make_identity

F32 = mybir.dt.float32
BF16 = mybir.dt.bfloat16
FP8 = mybir.dt.float8e4
AF = mybir.ActivationFunctionType
ALU = mybir.AluOpType
DR = mybir.MatmulPerfMode.DoubleRow

B, N, C = 2, 8192, 768
H, K = 12, 256
D = C // H                 # 64
HID = 4 * C                # 3072
EPS = 1e-6
NCORES = 8
T = (B * N) // NCORES      # 2048 tokens per core
NT = T // 128              # 16 token tiles
NG = T // 512              # 4 token groups
CK = C // 128              # 6 contraction chunks of C
HC = HID // 128            # 24 hidden chunks
KC = K // 128              # 2 kk chunks
SCALE = float(D) ** -0.5   # 0.125

SW = 128.0                 # host-side weight scale into fp8 range

_CACHE = {}


def _ln_stats(nc, pool, xt, tag):
    """LayerNorm stats for a (128, C) fp32 tile -> (rstd, -mu*rstd) (128,1)."""
    NSUB = 3  # 768 = 3 x 256 (BN_STATS_FMAX=512, gcd=256)
    stats = pool.tile([128, NSUB, 6], F32, tag=f"{tag}_stats")
    xv = xt.rearrange("p (j s) -> p j s", j=NSUB)
    for j in range(NSUB):
        nc.vector.bn_stats(stats[:, j, :], xv[:, j, :])
    mv = pool.tile([128, 2], F32, tag=f"{tag}_mv")
    nc.vector.bn_aggr(mv[:], stats[:])
    var = pool.tile([128, 1], F32, tag=f"{tag}_var")
    nc.vector.tensor_scalar_add(var[:], mv[:, 1:2], EPS)
    std = pool.tile([128, 1], F32, tag=f"{tag}_std")
    nc.scalar.activation(std[:], var[:], AF.Sqrt)
    rstd = pool.tile([128, 1], F32, tag=f"{tag}_rstd")
    nc.vector.reciprocal(rstd[:], std[:])
    nmr = pool.tile([128, 1], F32, tag=f"{tag}_nmr")
    nc.vector.scalar_tensor_tensor(nmr[:], mv[:, 0:1], -1.0, rstd[:],
                                   op0=ALU.mult, op1=ALU.mult)
    return rstd, nmr


def build(ln1_triv, ln2_triv, qb_zero, kvb_zero, projb_zero, fc2b_zero):
    nc = bacc.Bacc("TRN2", target_bir_lowering=False, debug=False,
                   enable_asserts=True, num_devices=NCORES)

    x_s = nc.dram_tensor("x_s", [T, C], F32, kind="ExternalInput").ap()
    qkv_w = nc.dram_tensor("qkv_w", [128, CK // 2, 2, 3 * C], FP8,
                           kind="ExternalInput").ap()
    qkv_b = nc.dram_tensor("qkv_b", [3 * C], F32, kind="ExternalInput").ap()
    Ek_s = nc.dram_tensor("Ek_s", [T, K], BF16, kind="ExternalInput").ap()
    Ev_s = nc.dram_tensor("Ev_s", [T, K], BF16, kind="ExternalInput").ap()
    proj_w = nc.dram_tensor("proj_w", [128, CK, C], BF16,
                            kind="ExternalInput").ap()
    proj_b = nc.dram_tensor("proj_b", [C], F32, kind="ExternalInput").ap()
    fc1_w = nc.dram_tensor("fc1_w", [128, CK // 2, 2, HID], FP8,
                           kind="ExternalInput").ap()
    fc1_b = nc.dram_tensor("fc1_b", [HID], F32, kind="ExternalInput").ap()
    fc2_w = nc.dram_tensor("fc2_w", [128, HC // 2, 2, C], FP8,
                           kind="ExternalInput").ap()
    fc2_b = nc.dram_tensor("fc2_b", [C], F32, kind="ExternalInput").ap()
    ln1_w = nc.dram_tensor("ln1_w", [1, C], F32, kind="ExternalInput").ap()
    ln1_b = nc.dram_tensor("ln1_b", [1, C], F32, kind="ExternalInput").ap()
    ln2_w = nc.dram_tensor("ln2_w", [1, C], F32, kind="ExternalInput").ap()
    ln2_b = nc.dram_tensor("ln2_b", [1, C], F32, kind="ExternalInput").ap()
    out = nc.dram_tensor("out", [T, C], F32, kind="ExternalOutput").ap()

    with tile.TileContext(nc) as tc:
      with tc.tile_pool(name="const", bufs=1) as constp, \
           tc.tile_pool(name="dram", bufs=1, space="DRAM") as dram:
        ident = constp.tile([128, 128], F32, tag="ident")
        make_identity(nc, ident)
        ones16 = constp.tile([128, 64], BF16, tag="ones16")
        nc.scalar.activation(ones16[:], ident[:, 0:64], AF.Copy,
                             bias=1.0, scale=0.0)
        nbias2 = constp.tile([128, 1], F32, tag="nbias2")
        nc.scalar.activation(nbias2[:], ident[:, 0:1], AF.Copy,
                             bias=-2.0, scale=0.0)
        qb_sb = constp.tile([128, CK], F32, tag="qb_sb")
        nc.sync.dma_start(
            qb_sb[:], qkv_b[None, 0:C].rearrange("o (m p) -> p (o m)", p=128))
        fc1b = constp.tile([128, HC], F32, tag="fc1b")
        nc.sync.dma_start(fc1b[:], fc1_b.rearrange("(m p) -> p m", p=128))

        def bcast_row(name, src_ap, width):
            row = constp.tile([1, width], F32, tag=f"{name}_row")
            nc.sync.dma_start(row[:], src_ap)
            bc = constp.tile([128, width], F32, tag=f"{name}_bc")
            nc.gpsimd.partition_broadcast(bc[:], row[:])
            return bc

        ln1w_bc = ln1b_bc = ln2w_bc = ln2b_bc = None
        kvb_bc = projb_bc = fc2b_bc = None
        if not ln1_triv:
            ln1w_bc = bcast_row("ln1w", ln1_w[:], C)
            ln1b_bc = bcast_row("ln1b", ln1_b[:], C)
        if not ln2_triv:
            ln2w_bc = bcast_row("ln2w", ln2_w[:], C)
            ln2b_bc = bcast_row("ln2b", ln2_b[:], C)
        if not kvb_zero:
            kvb_bc = bcast_row("kvb", qkv_b[None, C:3 * C], 2 * C)
        if not projb_zero:
            projb_bc = bcast_row("projb", proj_b[None, :], C)
        if not fc2b_zero:
            fc2b_bc = bcast_row("fc2b", fc2_b[None, :], C)

        ar_in = dram.tile([128, 2, 1536], F32)
        ar_out = dram.tile([128, 2, 1536], F32)

        # ===== Stage A: LN1, h1T (DMA xbar), k+Ek partials, qT; then v ====
        with tc.tile_pool(name="Apool", bufs=1) as Ap:
            h1T = Ap.tile([128, CK // 2, 2, T], FP8, tag="h1T")
            qT = Ap.tile([128, CK, T], BF16, tag="qT")
            qkvw_sb = Ap.tile([128, CK // 2, 2, 3 * C], FP8, tag="qkvw")
            nc.sync.dma_start(qkvw_sb[:], qkv_w)

            def kv_cvt(kvp, dst, bias_slice):
                """psum k/v (scaled SW) -> bf16 true scale, optional bias."""
                if bias_slice is None:
                    nc.vector.tensor_scalar_mul(dst, kvp[:], 1.0 / SW)
                else:
                    nc.vector.scalar_tensor_tensor(dst, kvp[:], 1.0 / SW,
                                                   bias_slice,
                                                   op0=ALU.mult, op1=ALU.add)

            with tc.tile_pool(name="A1", bufs=3) as wk, \
                 tc.tile_pool(name="psKV", bufs=2, space="PSUM") as psKV, \
                 tc.tile_pool(name="psQ", bufs=1, space="PSUM") as psQ:
                # ---- pass 1: LN1 + h1T + k partials + qT ----
                with tc.tile_pool(name="psK", bufs=1, space="PSUM") as psK:
                    kacc = psK.tile([128, 1536], F32, tag="kacc")
                    for i in range(NT):
                        xt = wk.tile([128, C], F32, tag="xt")
                        nc.sync.dma_start(xt[:], x_s[i * 128:(i + 1) * 128, :])
                        rstd, nmr = _ln_stats(nc, wk, xt, "ln1")
                        h1 = wk.tile([128, C], BF16, tag="h1")
                        nc.gpsimd.tensor_scalar(h1[:], xt[:], rstd[:], nmr[:],
                                                op0=ALU.mult, op1=ALU.add)
                        if ln1w_bc is not None:
                            h1f = wk.tile([128, C], F32, tag="h1f")
                            nc.vector.tensor_mul(h1f[:], h1[:], ln1w_bc[:])
                            nc.vector.tensor_add(h1f[:], h1f[:], ln1b_bc[:])
                            nc.vector.tensor_copy(h1[:], h1f[:])
                        h1Tb = wk.tile([128, CK, 128], BF16, tag="h1Tb")
                        nc.sync.dma_start(h1Tb[:], h1[:], transpose=True)
                        nc.gpsimd.tensor_copy(
                            h1T[:, :, :, i * 128:(i + 1) * 128].rearrange(
                                "p j s t -> p (j s) t"),
                            h1Tb[:])
                        # k = h1 @ Wk  (DoubleRow fp8 over c-chunk pairs)
                        kvp = psKV.tile([128, C], F32, tag="kvp")
                        for lo, hi in ((0, 512), (512, 768)):
                            for j in range(CK // 2):
                                nc.tensor.matmul(
                                    kvp[:, lo:hi],
                                    h1T[:, j, :, i * 128:(i + 1) * 128],
                                    qkvw_sb[:, j, :, C + lo:C + hi],
                                    start=(j == 0), stop=(j == CK // 2 - 1),
                                    perf_mode=DR)
                        kvk = wk.tile([128, C], BF16, tag="kvk")
                        kv_cvt(kvp, kvk[:],
                               None if kvb_bc is None else kvb_bc[:, 0:C])
                        Ekc = wk.tile([128, K], BF16, tag="Ekc")
                        nc.sync.dma_start(Ekc[:],
                                          Ek_s[i * 128:(i + 1) * 128, :])
                        st = (i == 0)
                        sp = (i == NT - 1)
                        for kc in range(KC):
                            for lo, hi in ((0, 512), (512, 768)):
                                nc.tensor.matmul(
                                    kacc[:, kc * 768 + lo:kc * 768 + hi],
                                    Ekc[:, kc * 128:(kc + 1) * 128],
                                    kvk[:, lo:hi], start=st, stop=sp)
                        # qT for group g once its 4 tiles are done
                        if i % 4 == 3:
                            g = i // 4
                            for m in range(CK):
                                qp = psQ.tile([128, 512], F32, tag="qp")
                                for j in range(CK // 2):
                                    nc.tensor.matmul(
                                        qp[:],
                                        qkvw_sb[:, j, :, m * 128:(m + 1) * 128],
                                        h1T[:, j, :, g * 512:(g + 1) * 512],
                                        start=(j == 0),
                                        stop=(j == CK // 2 - 1), perf_mode=DR)
                                nc.scalar.activation(
                                    qT[:, m, g * 512:(g + 1) * 512], qp[:],
                                    AF.Identity, scale=1.0 / SW,
                                    bias=(0.0 if qb_zero
                                          else qb_sb[:, m:m + 1]))
                    kacc_sb = wk.tile([128, 1536], F32, tag="acc_sb")
                    nc.scalar.activation(kacc_sb[:], kacc[:], AF.Copy)
                    nc.sync.dma_start(ar_in[:, 0, :], kacc_sb[:])
                # ---- pass 2: v partials (h1T already built) ----
                with tc.tile_pool(name="psV", bufs=1, space="PSUM") as psV:
                    vacc = psV.tile([128, 1536], F32, tag="vacc")
                    for i in range(NT):
                        kvp = psKV.tile([128, C], F32, tag="kvp")
                        for lo, hi in ((0, 512), (512, 768)):
                            for j in range(CK // 2):
                                nc.tensor.matmul(
                                    kvp[:, lo:hi],
                                    h1T[:, j, :, i * 128:(i + 1) * 128],
                                    qkvw_sb[:, j, :, 2 * C + lo:2 * C + hi],
                                    start=(j == 0), stop=(j == CK // 2 - 1),
                                    perf_mode=DR)
                        kvk = wk.tile([128, C], BF16, tag="kvk")
                        kv_cvt(kvp, kvk[:],
                               None if kvb_bc is None else kvb_bc[:, C:2 * C])
                        Ekc = wk.tile([128, K], BF16, tag="Ekc")
                        nc.sync.dma_start(Ekc[:],
                                          Ev_s[i * 128:(i + 1) * 128, :])
                        st = (i == 0)
                        sp = (i == NT - 1)
                        for kc in range(KC):
                            for lo, hi in ((0, 512), (512, 768)):
                                nc.tensor.matmul(
                                    vacc[:, kc * 768 + lo:kc * 768 + hi],
                                    Ekc[:, kc * 128:(kc + 1) * 128],
                                    kvk[:, lo:hi], start=st, stop=sp)
                    vacc_sb = wk.tile([128, 1536], F32, tag="acc_sb")
                    nc.scalar.activation(vacc_sb[:], vacc[:], AF.Copy)
                    nc.sync.dma_start(ar_in[:, 1, :], vacc_sb[:])

            nc.gpsimd.collective_compute(
                "AllReduce", ALU.add,
                replica_groups=[[0, 1, 2, 3], [4, 5, 6, 7]],
                ins=[ar_in.opt()], outs=[ar_out.opt()])

            # ===== post-AR: kT (transposed) and v_r, both bf16 =============
            with tc.tile_pool(name="kvp2", bufs=1) as kvp2:
                v_r = kvp2.tile([128, KC, C], BF16, tag="v_r")
                vf = kvp2.tile([128, 1536], F32, tag="vf")
                nc.sync.dma_start(vf[:], ar_out[:, 1, :])
                nc.scalar.activation(v_r[:].rearrange("p a b -> p (a b)"),
                                     vf[:], AF.Copy)
                kT = kvp2.tile([128, CK, K], BF16, tag="kT")
                with tc.tile_pool(name="kfp", bufs=1) as kfp:
                    kf = kfp.tile([128, 1536], F32, tag="kf")
                    nc.sync.dma_start(kf[:], ar_out[:, 0, :])
                    kfb = kfp.tile([128, 1536], BF16, tag="kfb")
                    nc.scalar.activation(kfb[:], kf[:], AF.Copy)
                    for kc in range(KC):
                        kTb = kfp.tile([128, CK, 128], BF16, tag="kTb")
                        nc.sync.dma_start(
                            kTb[:], kfb[:, kc * 768:(kc + 1) * 768],
                            transpose=True)
                        nc.gpsimd.tensor_copy(
                            kT[:, :, kc * 128:(kc + 1) * 128], kTb[:])

                # ===== Attention + proj + LN2 + MLP ========================
                with tc.tile_pool(name="at", bufs=2) as at, \
                     tc.tile_pool(name="pj", bufs=2) as pj, \
                     tc.tile_pool(name="pjx", bufs=2) as pjx, \
                     tc.tile_pool(name="ml", bufs=2) as ml, \
                     tc.tile_pool(name="ml1", bufs=1) as ml1, \
                     tc.tile_pool(name="psL", bufs=2, space="PSUM") as psL, \
                     tc.tile_pool(name="psN", bufs=2, space="PSUM") as psN, \
                     tc.tile_pool(name="psO", bufs=1, space="PSUM") as psO, \
                     tc.tile_pool(name="psP", bufs=1, space="PSUM") as psP, \
                     tc.tile_pool(name="psF", bufs=2, space="PSUM") as psF:
                    pw = ml1.tile([128, CK, C], BF16, tag="pw")
                    nc.sync.dma_start(pw[:], proj_w)
                    f1w = ml1.tile([128, CK // 2, 2, HID], FP8, tag="f1w")
                    nc.sync.dma_start(f1w[:], fc1_w)
                    f2w = ml1.tile([128, HC // 2, 2, C], FP8, tag="f2w")
                    nc.sync.dma_start(f2w[:], fc2_w)

                    for g in range(NG):
                        t0 = g * 512
                        # ---- attention for token group g ----
                        oT = at.tile([128, CK, 512], BF16, tag="oT")
                        for ph in range(H // 2):
                            eTs = []
                            rcbp = at.tile([128, 512], F32, tag=f"rcb{ph % 2}")
                            dn = psN.tile([128, 512], F32, tag="dn")
                            for sub in range(2):
                                h = 2 * ph + sub
                                off = 64 * (h % 2)
                                ch = h // 2
                                eT = at.tile([128, 2, 512], BF16,
                                             tag=f"eT{sub}")
                                for kc in range(KC):
                                    lg = psL.tile([128, 512], F32, tag="lg")
                                    nc.tensor.matmul(
                                        lg[:],
                                        kT[off:off + 64, ch,
                                           kc * 128:(kc + 1) * 128],
                                        qT[off:off + 64, ch, t0:t0 + 512],
                                        start=True, stop=True,
                                        tile_position=(off, 0))
                                    nc.scalar.activation(
                                        eT[:, kc, :], lg[:], AF.Exp,
                                        scale=SCALE, bias=nbias2[:])
                                for kc in range(KC):
                                    nc.tensor.matmul(
                                        dn[off:off + 64, :],
                                        ones16[:], eT[:, kc, :],
                                        start=(kc == 0), stop=(kc == KC - 1),
                                        tile_position=(0, off))
                                eTs.append(eT)
                            nc.vector.reciprocal(rcbp[:], dn[:])
                            # o^T for the head pair (plain bf16)
                            pav = psO.tile([128, 512], F32, tag="pav")
                            for sub in range(2):
                                off = 64 * sub
                                for kc in range(KC):
                                    nc.tensor.matmul(
                                        pav[off:off + 64, :],
                                        v_r[:, kc, ph * 128 + off:
                                            ph * 128 + off + 64],
                                        eTs[sub][:, kc, :],
                                        start=(kc == 0), stop=(kc == KC - 1),
                                        tile_position=(0, off))
                            nc.vector.tensor_tensor(
                                oT[:, ph, :], pav[:], rcbp[:], op=ALU.mult)

                        # ---- proj + residual + LN2 + h2T for group g ----
                        h2T = pj.tile([128, CK // 2, 2, 512], FP8, tag="h2T")
                        x2g = pjx.tile([128, 4, C], F32, tag="x2g")
                        for ms in range(4):
                            r0 = t0 + ms * 128
                            xr = pj.tile([128, C], F32, tag="xr")
                            nc.sync.dma_start(xr[:], x_s[r0:r0 + 128, :])
                            for cs in range(2):
                                pp = psP.tile([128, 384], F32, tag="pp")
                                for k in range(CK):
                                    nc.tensor.matmul(
                                        pp[:],
                                        oT[:, k, ms * 128:(ms + 1) * 128],
                                        pw[:, k, cs * 384:(cs + 1) * 384],
                                        start=(k == 0), stop=(k == CK - 1))
                                nc.vector.tensor_add(
                                    x2g[:, ms, cs * 384:(cs + 1) * 384],
                                    pp[:], xr[:, cs * 384:(cs + 1) * 384])
                            if projb_bc is not None:
                                nc.vector.tensor_add(x2g[:, ms, :],
                                                     x2g[:, ms, :], projb_bc[:])
                            rstd2, nmr2 = _ln_stats(nc, pj, x2g[:, ms, :],
                                                    f"ln2_{ms % 2}")
                            h2 = pj.tile([128, C], BF16, tag="h2")
                            nc.gpsimd.tensor_scalar(h2[:], x2g[:, ms, :],
                                                    rstd2[:], nmr2[:],
                                                    op0=ALU.mult, op1=ALU.add)
                            if ln2w_bc is not None:
                                h2f = pj.tile([128, C], F32, tag="h2f")
                                nc.vector.tensor_mul(h2f[:], h2[:], ln2w_bc[:])
                                nc.vector.tensor_add(h2f[:], h2f[:],
                                                     ln2b_bc[:])
                                nc.vector.tensor_copy(h2[:], h2f[:])
                            h2Tb = pj.tile([128, CK, 128], BF16, tag="h2Tb")
                            nc.sync.dma_start(h2Tb[:], h2[:], transpose=True)
                            nc.gpsimd.tensor_copy(
                                h2T[:, :, :, ms * 128:(ms + 1) * 128].rearrange(
                                    "p j s t -> p (j s) t"),
                                h2Tb[:])

                        # ---- MLP for group g (fp8 DoubleRow) ----
                        gT = ml.tile([128, HC // 2, 2, 512], FP8, tag="gT")
                        for hc in range(HC):
                            fp = psF.tile([128, 512], F32, tag="fp")
                            for j in range(CK // 2):
                                nc.tensor.matmul(
                                    fp[:],
                                    f1w[:, j, :, hc * 128:(hc + 1) * 128],
                                    h2T[:, j, :, :], start=(j == 0),
                                    stop=(j == CK // 2 - 1), perf_mode=DR)
                            nc.scalar.activation(gT[:, hc // 2, hc % 2, :],
                                                 fp[:], AF.Gelu,
                                                 scale=1.0 / SW,
                                                 bias=fc1b[:, hc:hc + 1])
                        for cs in range(2):
                            for ms in range(4):
                                r0 = t0 + ms * 128
                                op = psF.tile([128, 512], F32, tag="fp")
                                for hp in range(HC // 2):
                                    nc.tensor.matmul(
                                        op[:, 0:384],
                                        gT[:, hp, :, ms * 128:(ms + 1) * 128],
                                        f2w[:, hp, :, cs * 384:(cs + 1) * 384],
                                        start=(hp == 0),
                                        stop=(hp == HC // 2 - 1), perf_mode=DR)
                                oth = ml.tile([128, 384], F32, tag="oth")
                                nc.vector.scalar_tensor_tensor(
                                    oth[:], op[:, 0:384], 1.0 / SW,
                                    x2g[:, ms, cs * 384:(cs + 1) * 384],
                                    op0=ALU.mult, op1=ALU.add)
                                if fc2b_bc is not None:
                                    nc.vector.tensor_add(
                                        oth[:], oth[:],
                                        fc2b_bc[:, cs * 384:(cs + 1) * 384])
                                nc.sync.dma_start(
                                    out[r0:r0 + 128, cs * 384:(cs + 1) * 384],
                                    oth[:])

    nc.compile()
    return nc


def _to_fp8(a):
    return np.ascontiguousarray(a.astype(ml_dtypes.float8_e4m3))


def _w_fp8(w):
    """[C_in, C_out] -> [128, C_in/256, 2, C_out] fp8, scaled by SW."""
    ci, co = w.shape
    r = (w * SW).reshape(ci // 256, 2, 128, co).transpose(2, 0, 1, 3)
    return _to_fp8(np.ascontiguousarray(r))


def kernel(**inputs):
    x = np.ascontiguousarray(np.asarray(inputs["x"], dtype=np.float32))
    qkv_w = np.asarray(inputs["qkv_w"], dtype=np.float32)
    qkv_b = np.ascontiguousarray(np.asarray(inputs["qkv_b"], dtype=np.float32))
    Ek = np.asarray(inputs["Ek"], dtype=np.float32)
    Ev = np.asarray(inputs["Ev"], dtype=np.float32)
    proj_w = np.asarray(inputs["proj_w"], dtype=np.float32)
    proj_b = np.ascontiguousarray(np.asarray(inputs["proj_b"], dtype=np.float32))
    fc1_w = np.asarray(inputs["fc1_w"], dtype=np.float32)
    fc1_b = np.ascontiguousarray(np.asarray(inputs["fc1_b"], dtype=np.float32))
    fc2_w = np.asarray(inputs["fc2_w"], dtype=np.float32)
    fc2_b = np.ascontiguousarray(np.asarray(inputs["fc2_b"], dtype=np.float32))
    ln1_w = np.asarray(inputs["ln1_w"], dtype=np.float32)
    ln1_b = np.asarray(inputs["ln1_b"], dtype=np.float32)
    ln2_w = np.asarray(inputs["ln2_w"], dtype=np.float32)
    ln2_b = np.asarray(inputs["ln2_b"], dtype=np.float32)

    ln1_triv = bool(np.all(ln1_w == 1.0) and np.all(ln1_b == 0.0))
    ln2_triv = bool(np.all(ln2_w == 1.0) and np.all(ln2_b == 0.0))
    qb_zero = bool(np.all(qkv_b[0:C] == 0.0))
    kvb_zero = bool(np.all(qkv_b[C:] == 0.0))
    projb_zero = bool(np.all(proj_b == 0.0))
    fc2b_zero = bool(np.all(fc2_b == 0.0))

    key = (ln1_triv, ln2_triv, qb_zero, kvb_zero, projb_zero, fc2b_zero)
    if key not in _CACHE:
        _CACHE[key] = build(*key)
    nc = _CACHE[key]

    qkv_w8 = _w_fp8(qkv_w)
    fc1_w8 = _w_fp8(fc1_w)
    fc2_w8 = _w_fp8(fc2_w)
    proj_wb = np.ascontiguousarray(
        proj_w.reshape(CK, 128, C).transpose(1, 0, 2).astype(
            ml_dtypes.bfloat16))

    def ek_prep(E, pos0):
        return np.ascontiguousarray(
            E[pos0:pos0 + T].astype(ml_dtypes.bfloat16))

    xf = x.reshape(B * N, C)
    in_maps = []
    for c in range(NCORES):
        pos0 = (c % 4) * T
        in_maps.append({
            "x_s": np.ascontiguousarray(xf[c * T:(c + 1) * T]),
            "qkv_w": qkv_w8,
            "qkv_b": qkv_b,
            "Ek_s": ek_prep(Ek, pos0),
            "Ev_s": ek_prep(Ev, pos0),
            "proj_w": proj_wb,
            "proj_b": proj_b,
            "fc1_w": fc1_w8,
            "fc1_b": fc1_b,
            "fc2_w": fc2_w8,
            "fc2_b": fc2_b,
            "ln1_w": np.ascontiguousarray(ln1_w.reshape(1, C)),
            "ln1_b": np.ascontiguousarray(ln1_b.reshape(1, C)),
            "ln2_w": np.ascontiguousarray(ln2_w.reshape(1, C)),
            "ln2_b": np.ascontiguousarray(ln2_b.reshape(1, C)),
        })

    import os
    trace = bool(os.environ.get("NN_BLOCK_TRACE"))
    res = run_bass_kernel_spmd(nc, in_maps, core_ids=list(range(NCORES)),
                               trace=trace)
    global LAST_RESULT
    LAST_RESULT = res
    outs = np.concatenate([res.results[c]["out"] for c in range(NCORES)],
                          axis=0)
    return outs.reshape(B, N, C)


LAST_RESULT = None
